# revision 5
# baseline (speedup 1.0000x reference)
"""Masked L1 loss (anomaly VQ loss) on 8 Trainium2 NeuronCores.

reference math:
    num = sum(|pred - vq[c]| * (1 - mask))   over (N,V,C,T,H,W)
    den = sum(1 - mask) * V*C*T              (mask broadcast over V,C,T)
    out = num / den

Sharding: data-parallel over the batch axis N=8 -> one batch element per core.

Per-core pipeline (fp8 end-to-end), built around the identity
    |d| = d - 2*min(d, 0),  d = x - vq[c]
so that most elements need only ONE elementwise op (DVE's 2-stage ALU fuses
subtract+min in a single 1814ns pass; a plain 2-op abs costs 2779ns):

  pred: host-cast to float8_e4m3 (HBM traffic /4; kernel is HBM-read bound)
  and laid out c-pair-major [12, 128, 6144] so each DMA is one contiguous
  6KB-per-partition transfer.  p = t*16+(h>>3), f = (h&7)*128+w, s = c2*3+v.

  Lanes, assigned per c-pair:
   d1 (DVE): mt = min(x - vq_c, 0) in fp8 (one op per c-group).  PE then
      accumulates BOTH the raw slabs (stationary [I|I], DoubleRow fp8 - two
      slabs per pass) and the min slabs (stationary [-2I|-2I]); PSUM gets
      sum(x) - 2*sum(min) = sum(|d| + vq_c).  The constant vq_c*sum(wm)
      surplus is subtracted on the host (it only depends on vq and wsum).
   act (ACT): at = Abs(-x + vq_c) fused, fp8 out; PE accumulates DR pairs
      with [I|I].

  mask: uploaded as fp8 {0,1} [16,1024]; 8 scalar-ring DMAs T-replicate it
  to [128,1024]; wm = 1-m and the wsum reduction run early on DVE, before
  the first pred DMA even lands.

  epilogue: per PSUM half, one fused DVE op
    scalar_tensor_tensor(junk, acc_h, 1.0, wm_h, mult, mult,
                         accum_out=r4[:,h]) ,
  with the last c-pair's matmuls ordered half0-first so the half-0 epilogue
  overlaps the half-1 matmuls.  Host combines:
    num = sum(r4[:,0:2]) - 3*S_wm*sum(vq_c over d1 pairs)
    out = num / (V*C * S_wm)
"""

import os
import sys

for _p in ("/opt/trn_rl_repo", "/root/.axon_site/_ro/trn_rl_repo"):
    if os.path.isdir(_p) and _p not in sys.path:
        sys.path.insert(0, _p)

import numpy as np

import concourse.bacc as bacc
import concourse.mybir as mybir
import concourse.tile as tile
from concourse.bass_utils import run_bass_kernel_spmd

N_CORES = 8
V, C, T, H, W = 3, 24, 8, 128, 128
P = 128
FD = T * W          # 1024 free elements per slab
S = 2 * V           # 6 slabs per c-pair
CP = C // 2         # 12 c-pairs
HALF = FD // 2      # 512 = one PSUM bank of fp32

F32 = mybir.dt.float32
BF16 = mybir.dt.bfloat16
FP8 = mybir.dt.float8e4

ALU = mybir.AluOpType
ACTF = mybir.ActivationFunctionType
DR = mybir.MatmulPerfMode.DoubleRow

# lane per c-pair: "d1" = DVE min-trick, "act" = ACT fused-abs.
# 7 d1 / 5 act balances DVE(1814/grp) vs ACT(2838/grp); spread so neither
# lane falls behind the DMA stream; the last two pairs are d1 because DVE
# drains fastest (slab-granular min ops chasing the final DMAs).
LANES = ("d1", "act", "d1", "act", "d1", "act", "d1", "act", "d1", "act", "d1", "d1")


def build_nc(lanes=LANES):
    nc = bacc.Bacc("TRN2", target_bir_lowering=False, debug=False)

    pred = nc.declare_dram_parameter("pred", [CP, P, S * FD], FP8, isOutput=False)
    m_d = nc.declare_dram_parameter("m_host", [16, FD], FP8, isOutput=False)
    vqb_d = nc.declare_dram_parameter("vqb_host", [P, C], F32, isOutput=False)
    # stationaries: [I|I] and [-2I|-2I], fp8, as [128, 2, 128]
    wpos_d = nc.declare_dram_parameter("wpos_host", [P, 2 * P], FP8, isOutput=False)
    wneg_d = nc.declare_dram_parameter("wneg_host", [P, 2 * P], FP8, isOutput=False)
    out = nc.declare_dram_parameter("out", [P, 4], F32, isOutput=True)

    with tile.TileContext(nc) as tc:
        with (
            tc.tile_pool(name="const", bufs=1) as constp,
            tc.tile_pool(name="predp", bufs=CP) as predp,
            tc.tile_pool(name="absp", bufs=5) as absp,
            tc.tile_pool(name="psum", bufs=1, space="PSUM") as psump,
            tc.tile_pool(name="fin", bufs=1) as finp,
        ):
            # --- constants + mask ride SWDGE (gpsimd): zero tax on the sync
            # pred stream and on the ACT compute lane
            vqb = constp.tile([P, C], F32)
            wpos = constp.tile([P, 2, P], FP8)
            wneg = constp.tile([P, 2, P], FP8)
            m8 = finp.tile([P, FD], FP8)
            nc.gpsimd.dma_start(vqb[:, :], vqb_d[:, :])
            nc.gpsimd.dma_start(wpos[:, :, :], wpos_d[:, :].rearrange("p (two m) -> p two m", two=2))
            nc.gpsimd.dma_start(wneg[:, :, :], wneg_d[:, :].rearrange("p (two m) -> p two m", two=2))
            for t in range(T):
                nc.gpsimd.dma_start(m8[16 * t : 16 * (t + 1), :], m_d[:, :])

            # --- queue every pred DMA up front: sync HWDGE ring stays full
            pts = []
            for cp in range(CP):
                pt = predp.tile([P, S, FD], FP8, tag="pt")
                if cp == 0:
                    # first pair split per-group: compute starts ~1.2us sooner
                    nc.sync.dma_start(pt[:, 0:3, :], pred[cp][:, : 3 * FD])
                    nc.sync.dma_start(pt[:, 3:6, :], pred[cp][:, 3 * FD :])
                elif cp == CP - 1:
                    # last pair per slab-pair: short tail, 3 triggers
                    for q in range(3):
                        nc.sync.dma_start(
                            pt[:, 2 * q : 2 * q + 2, :],
                            pred[cp][:, 2 * q * FD : (2 * q + 2) * FD],
                        )
                else:
                    nc.sync.dma_start(pt[:, :, :], pred[cp])
                pts.append(pt)

            acc = psump.tile([P, FD], F32)
            r4 = finp.tile([P, 4], F32)
            wm = finp.tile([P, FD], BF16)

            # early, off the critical tail: wm = 1 - m, wsum partial
            nc.vector.memset(r4[:, 3:4], 0.0)
            nc.vector.tensor_scalar(
                wm[:, :], m8[:, :], -1.0, 1.0, op0=ALU.mult, op1=ALU.add
            )
            nc.vector.tensor_reduce(
                r4[:, 2:3], wm[:, :], axis=mybir.AxisListType.X, op=ALU.add
            )

            # matmul start/stop bookkeeping per PSUM half
            n_mm = [0, 0]
            total_mm = [0, 0]
            for cp in range(CP):
                total_mm[0] += 6 if lanes[cp] == "d1" else 3
                total_mm[1] += 6 if lanes[cp] == "d1" else 3

            def mm(h, stat, rhs_t, q):
                first = n_mm[h] == 0
                n_mm[h] += 1
                last = n_mm[h] == total_mm[h]
                nc.tensor.matmul(
                    acc[:, h * HALF : (h + 1) * HALF],
                    stat[:, :, :],
                    rhs_t[:, 2 * q : 2 * q + 2, h * HALF : (h + 1) * HALF],
                    start=first, stop=last, perf_mode=DR,
                )

            # --- main loop over c-pairs -----------------------------------
            for cp in range(CP):
                pt = pts[cp]
                last_cp = cp == CP - 1
                h_order = (0, 1)
                if lanes[cp] == "act":
                    at = absp.tile([P, S, FD], FP8, tag="at")
                    for c2 in (0, 1):
                        c = 2 * cp + c2
                        nc.scalar.activation(
                            at[:, 3 * c2 : 3 * (c2 + 1), :],
                            pt[:, 3 * c2 : 3 * (c2 + 1), :],
                            ACTF.Abs, bias=vqb[:, c : c + 1], scale=-1.0,
                        )
                    for h in h_order:
                        for q in range(3):
                            mm(h, wpos, at, q)
                else:
                    # d1: raw slabs accumulate straight off the DMA
                    for h in h_order:
                        for q in range(3):
                            mm(h, wpos, pt, q)
                    mt = absp.tile([P, S, FD], FP8, tag="at")
                    if last_cp:
                        # slab-granular min ops chase the per-slab DMAs
                        for s in range(S):
                            c = 2 * cp + (s // 3)
                            nc.vector.tensor_scalar(
                                mt[:, s, :], pt[:, s, :], vqb[:, c : c + 1], 0.0,
                                op0=ALU.subtract, op1=ALU.min,
                            )
                    else:
                        for c2 in (0, 1):
                            c = 2 * cp + c2
                            nc.vector.tensor_scalar(
                                mt[:, 3 * c2 : 3 * (c2 + 1), :],
                                pt[:, 3 * c2 : 3 * (c2 + 1), :],
                                vqb[:, c : c + 1], 0.0,
                                op0=ALU.subtract, op1=ALU.min,
                            )
                    if last_cp:
                        # half 0 fully finished first so its epilogue can
                        # overlap half 1's matmuls
                        for h in (0, 1):
                            for q in range(3):
                                mm(h, wneg, mt, q)
                            if h == 0:
                                junk0 = finp.tile([P, HALF], BF16)
                                nc.vector.scalar_tensor_tensor(
                                    junk0[:, :], acc[:, :HALF], 1.0, wm[:, :HALF],
                                    op0=ALU.mult, op1=ALU.mult,
                                    accum_out=r4[:, 0:1],
                                )
                    else:
                        for h in h_order:
                            for q in range(3):
                                mm(h, wneg, mt, q)

            junk1 = finp.tile([P, HALF], BF16)
            nc.vector.scalar_tensor_tensor(
                junk1[:, :], acc[:, HALF:], 1.0, wm[:, HALF:],
                op0=ALU.mult, op1=ALU.mult, accum_out=r4[:, 1:2],
            )
            nc.sync.dma_start(out[:, :], r4[:, :])

    nc.compile()
    return nc


_NC_CACHE = None


def _get_nc():
    global _NC_CACHE
    if _NC_CACHE is None:
        _NC_CACHE = build_nc()
    return _NC_CACHE


def make_in_maps(pred, mask_extreme, vq_0):
    import ml_dtypes

    fp8 = ml_dtypes.float8_e4m3fn
    pred8 = np.asarray(pred).astype(fp8)
    # (N,V,C,T,H,W) -> per core [cp, p, c2, v, f] contiguous
    x = pred8.reshape(N_CORES, V, C, P, FD)
    x = x.transpose(0, 2, 3, 1, 4)                  # (N, C, P, V, FD)
    x = x.reshape(N_CORES, CP, 2, P, V, FD)
    x = np.ascontiguousarray(x.transpose(0, 1, 3, 2, 4, 5))  # (N, CP, P, 2, V, FD)
    x = x.reshape(N_CORES, CP, P, S * FD)

    m_host = np.asarray(mask_extreme, dtype=np.int32).astype(fp8)
    m_host = m_host.reshape(N_CORES, 16, FD)

    vq_0 = np.ascontiguousarray(vq_0, dtype=np.float32)
    vqb = np.ascontiguousarray(np.tile(vq_0, (P, 1)))
    eye = np.eye(P, dtype=np.float32)
    wpos = np.ascontiguousarray(np.concatenate([eye, eye], axis=1).astype(fp8))
    wneg = np.ascontiguousarray(np.concatenate([-2 * eye, -2 * eye], axis=1).astype(fp8))

    in_maps = []
    for i in range(N_CORES):
        in_maps.append(
            {
                "pred": x[i],
                "m_host": m_host[i],
                "vqb_host": vqb,
                "wpos_host": wpos,
                "wneg_host": wneg,
            }
        )
    return in_maps


# host-side vq correction: d1 lanes accumulate sum(x) - 2*sum(min) whose
# wm-dot exceeds sum(wm*|d|) by vq_c * S_wm per slab (3 slabs per c-group)
D1_CS = [c for cp in range(CP) if LANES[cp] == "d1" for c in (2 * cp, 2 * cp + 1)]


def combine(results, vq_0):
    vq64 = np.asarray(vq_0, dtype=np.float64).reshape(-1)
    vq_d1 = float(vq64[D1_CS].sum())
    num = 0.0
    wsum = 0.0
    for r in results:
        o = np.asarray(r["out"], dtype=np.float64)  # [128, 4] per-partition partials
        s_wm = o[:, 2].sum()
        num += o[:, 0].sum() + o[:, 1].sum() - 3.0 * s_wm * vq_d1
        wsum += s_wm
    den = wsum * float(V * C)  # wsum already counts each mask element T times
    return np.array(num / den, dtype=np.float32)


def kernel(pred, mask_extreme, vq_0):
    nc = _get_nc()
    in_maps = make_in_maps(pred, mask_extreme, vq_0)
    res = run_bass_kernel_spmd(nc, in_maps, core_ids=list(range(N_CORES)))
    return combine(res.results, vq_0)


if __name__ == "__main__":
    rng = np.random.default_rng(0)
    pred = rng.standard_normal((8, V, C, T, H, W), dtype=np.float32)
    mask = rng.integers(0, 2, size=(8, H, W)).astype(np.int32)
    vq = rng.standard_normal((1, C), dtype=np.float32)
    got = kernel(pred=pred, mask_extreme=mask, vq_0=vq)
    m = mask.astype(np.float64)[:, None, None, None, :, :]
    w = 1.0 - m
    p64 = pred.astype(np.float64)
    numr = np.abs(p64 - vq.astype(np.float64)[0][None, None, :, None, None, None]) * w
    exp = numr.sum() / (w.sum() * V * C * T)
    print("kernel:", got, "expected:", exp, "rel:", abs(got - exp) / abs(exp))
